# revision 8
# baseline (speedup 1.0000x reference)
"""Trainium2 Bass kernel for nn_Adapter_SelfParam_CrossNonParam.

Bottleneck adapter: down-proj(1024->256)+exact GELU, self-attention over
the first 200 prompt tokens (4 heads), parameter-free cross-attention
prompt->tokens, concat, up-proj(256->1024), gate.

Sharding: data-parallel over batch B=64 across 8 NeuronCores (8 items
each); all weights replicated. No collectives.

V2: full bf16 datapath (operand rounding ~2e-3, PSUM accumulation fp32).
x arrives as bf16 (padded to 1280 tokens) and is transposed by the DMA
xbar engines straight from DRAM into feature-major SBUF tiles - no PE
transposes for x. All intermediates are feature-major so matmul
contractions land on partitions; the few small transposes (tok-natural,
P^T, v, P'^T, prompt_out^T) run on the PE at 1 cyc/row in bf16.
"""
import sys

sys.path.insert(0, "/opt/trn_rl_repo")

import numpy as np
import ml_dtypes
from contextlib import ExitStack

import concourse.bass as bass
import concourse.tile as tile
from concourse import bacc, mybir
from concourse.bass_utils import run_bass_kernel_spmd

F32 = mybir.dt.float32
BF16 = mybir.dt.bfloat16
AF = mybir.ActivationFunctionType

B, NTOK, C = 64, 1224, 1024
NPAD = 1280                 # token dim padded for DMA-transpose alignment
E, P, T = 256, 200, 1024
NH, HD = 4, 64
NCORES, BL = 8, 8           # cores, batch per core
ATT_SCALE = 1.0 / np.sqrt(HD)   # folded into q weights host-side
CROSS_SCALE = float(E) ** -0.5  # folded into cross-softmax exp scale

# prompt chunks (rows of the 200-token prompt)
PCH = [(0, 128), (128, 72)]
# down-proj N chunks
DNCH = [(0, 512), (512, 456), (968, 256)]


def build_nc():
    nc = bacc.Bacc("TRN2", target_bir_lowering=False, debug=False,
                   num_devices=NCORES)

    x_d = nc.dram_tensor("xb", [BL, NPAD, C], BF16, kind="ExternalInput").ap()
    dwT_d = nc.dram_tensor("dwT", [128, 2048], F32, kind="ExternalInput").ap()
    ipwt_d = nc.dram_tensor("ipwt", [2, 128, 768], F32, kind="ExternalInput").ap()
    opwt_d = nc.dram_tensor("opwt", [2, 128, 256], F32, kind="ExternalInput").ap()
    upwt_d = nc.dram_tensor("upwt", [2, 128, 1024], F32, kind="ExternalInput").ap()
    ident_d = nc.dram_tensor("ident", [128, 128], F32, kind="ExternalInput").ap()
    dbias_d = nc.dram_tensor("dbias", [128, 2], F32, kind="ExternalInput").ap()
    qkvb_d = nc.dram_tensor("qkvb", [128, 6], F32, kind="ExternalInput").ap()
    opb_d = nc.dram_tensor("opb", [128, 2], F32, kind="ExternalInput").ap()
    g_d = nc.dram_tensor("g128", [128, 1], F32, kind="ExternalInput").ap()
    out_d = nc.dram_tensor("out", [BL, NTOK, C], F32, kind="ExternalOutput").ap()

    with tile.TileContext(nc) as tc, ExitStack() as ctx:
        wp = ctx.enter_context(tc.tile_pool(name="wts", bufs=1))
        sb1 = ctx.enter_context(tc.tile_pool(name="sb1", bufs=1))
        sbx = ctx.enter_context(tc.tile_pool(name="sbx", bufs=2))
        pst = ctx.enter_context(tc.tile_pool(name="pst", bufs=8))
        pout = ctx.enter_context(tc.tile_pool(name="pout", bufs=3))
        pool_mm = ctx.enter_context(tc.tile_pool(name="pmm", bufs=3, space="PSUM"))
        pool_sm = ctx.enter_context(tc.tile_pool(name="psm", bufs=2, space="PSUM"))

        # ---- resident weights (bf16 via gpsimd cast-DMA) ----
        dwT = wp.tile([128, 2048], BF16, tag="dwT")
        nc.gpsimd.dma_start(dwT[:], dwT_d[:])
        ipWT = []
        for ki in range(2):
            t = wp.tile([128, 768], BF16, tag=f"ipWT{ki}", name=f"ipWT{ki}")
            nc.gpsimd.dma_start(t[:], ipwt_d[ki])
            ipWT.append(t)
        opWT = []
        for ki in range(2):
            t = wp.tile([128, 256], BF16, tag=f"opWT{ki}", name=f"opWT{ki}")
            nc.gpsimd.dma_start(t[:], opwt_d[ki])
            opWT.append(t)
        upWT = []
        for ki in range(2):
            t = wp.tile([128, 1024], BF16, tag=f"upWT{ki}", name=f"upWT{ki}")
            nc.gpsimd.dma_start(t[:], upwt_d[ki])
            upWT.append(t)
        idB = wp.tile([128, 128], BF16, tag="idB")
        nc.gpsimd.dma_start(idB[:], ident_d[:])
        dbias = wp.tile([128, 2], F32, tag="dbias")
        nc.sync.dma_start(dbias[:], dbias_d[:])
        qkvb = wp.tile([128, 6], F32, tag="qkvb")
        nc.sync.dma_start(qkvb[:], qkvb_d[:])
        opb = wp.tile([128, 2], F32, tag="opb")
        nc.sync.dma_start(opb[:], opb_d[:])
        g128 = wp.tile([128, 1], F32, tag="g128")
        nc.sync.dma_start(g128[:], g_d[:])

        # prompt^T for all 8 batch items: [256 feat, 8*200]
        promT = [wp.tile([128, BL * P], BF16, tag=f"promT{m}", name=f"promT{m}")
                 for m in range(2)]

        def transpose(out_ap, in_ap):
            pw = in_ap.partition_size()
            bp = in_ap.base_partition()
            nc.tensor.transpose(out_ap, in_ap, idB[bp:bp + pw, bp:bp + pw])

        for pair in range(BL // 2):
            tokT = {}   # (b01, m) -> [128, 1024] tok^T e-chunk
            tokN = {}   # (b01, tt) -> [128, 256] tok natural
            for b01 in range(2):
                b = pair * 2 + b01
                # ---- phase 1: DMA-transpose x, down-proj + gelu ----
                xT = [sbx.tile([128, NPAD], BF16, tag=f"xT{ci}",
                               name=f"xT{ci}") for ci in range(8)]
                for ci in range(8):
                    nc.sync.dma_start(
                        out=xT[ci][:],
                        in_=x_d[b, :, ci * 128:(ci + 1) * 128],
                        transpose=True)
                for m in range(2):
                    tokTm = sb1.tile([128, T], BF16, tag=f"tokT{b01}_{m}",
                                     name=f"tokT{b01}_{m}")
                    tokT[(b01, m)] = tokTm
                    for (n0, nw) in DNCH:
                        pmm = pool_mm.tile([128, 512], F32, tag="mm")
                        for ci in range(8):
                            nc.tensor.matmul(
                                pmm[:, :nw],
                                dwT[:, ci * 256 + m * 128:ci * 256 + (m + 1) * 128],
                                xT[ci][:, n0:n0 + nw],
                                start=(ci == 0), stop=(ci == 7))
                        if n0 == 0:
                            nc.scalar.activation(
                                promT[m][:, b * P:(b + 1) * P], pmm[:, :P],
                                AF.Gelu, bias=dbias[:, m:m + 1])
                            nc.scalar.activation(
                                tokTm[:, 0:nw - P], pmm[:, P:nw],
                                AF.Gelu, bias=dbias[:, m:m + 1])
                        else:
                            nc.scalar.activation(
                                tokTm[:, n0 - P:n0 - P + nw], pmm[:, :nw],
                                AF.Gelu, bias=dbias[:, m:m + 1])
                # tok natural layout for cross-attention second matmul
                for tt in range(8):
                    psm = pool_sm.tile([128, 256], BF16, tag="sm")
                    for m in range(2):
                        transpose(psm[:, m * 128:(m + 1) * 128],
                                  tokT[(b01, m)][:, tt * 128:(tt + 1) * 128])
                    tokNt = sb1.tile([128, 257], BF16, tag=f"tokN{b01}_{tt}",
                                     name=f"tokN{b01}_{tt}")
                    nc.vector.tensor_copy(tokNt[:, :256], psm[:])
                    nc.vector.memset(tokNt[:, 256:257], 1.0)
                    tokN[(b01, tt)] = tokNt

            # ---- phase 2: self-attention over prompts (pair) ----
            qkvT = []
            for mi in range(6):
                pmm = pool_mm.tile([128, 512], F32, tag="mm")
                for ki in range(2):
                    nc.tensor.matmul(
                        pmm[:, :2 * P],
                        ipWT[ki][:, mi * 128:(mi + 1) * 128],
                        promT[ki][:, pair * 2 * P:pair * 2 * P + 2 * P],
                        start=(ki == 0), stop=(ki == 1))
                qt = sb1.tile([128, 2 * P], BF16, tag=f"qkvT{mi}",
                              name=f"qkvT{mi}")
                nc.scalar.activation(qt[:], pmm[:, :2 * P], AF.Identity,
                                     bias=qkvb[:, mi:mi + 1])
                qkvT.append(qt)
            saIn = [sb1.tile([128, 2 * P], BF16, tag=f"saIn{ki}",
                             name=f"saIn{ki}") for ki in range(2)]
            for b01 in range(2):
                boff = b01 * P
                for mi in range(2):
                    qv, kv, vv = qkvT[mi], qkvT[2 + mi], qkvT[4 + mi]
                    # v natural layout, both heads of this chunk at once
                    vNc = []
                    for (k0, kw) in PCH:
                        psv = pool_sm.tile([128, 256], BF16, tag="sm")
                        transpose(psv[:kw, :128],
                                  vv[:, boff + k0:boff + k0 + kw])
                        vn = pst.tile([128, 128], BF16, tag="vNc")
                        nc.vector.tensor_copy(vn[:kw, :], psv[:kw, :128])
                        vNc.append(vn)
                    # scores + softmax; the two heads use disjoint PE row
                    # groups (base partition 0 / 64) and pack concurrently
                    Pn = {}
                    for (q0, qw) in PCH:
                        pss = []
                        for hh in range(2):
                            hoff = hh * 64
                            ps = pool_sm.tile([128, 200], F32, tag="ss",
                                              bufs=3)
                            nc.tensor.matmul(
                                ps[:qw, :P],
                                qv[hoff:hoff + 64, boff + q0:boff + q0 + qw],
                                kv[hoff:hoff + 64, boff:boff + P],
                                start=True, stop=True)
                            pss.append(ps)
                        for hh in range(2):
                            pe = pst.tile([128, P], BF16, tag="Pexp")
                            se = pst.tile([128, 1], F32, tag="sexp")
                            nc.scalar.activation(pe[:qw], pss[hh][:qw, :P],
                                                 AF.Exp, bias=0.0,
                                                 accum_out=se[:qw])
                            rc = pst.tile([128, 1], F32, tag="recip")
                            nc.vector.reciprocal(rc[:qw], se[:qw])
                            pn = pst.tile([128, P], BF16, tag="Pn")
                            nc.vector.tensor_scalar_mul(pn[:qw], pe[:qw],
                                                        rc[:qw])
                            Pn[(hh, q0)] = pn
                    for hh in range(2):
                        hoff = hh * 64
                        PTs = []
                        for (k0, kw) in PCH:
                            psm = pool_sm.tile([128, 256], BF16, tag="sm")
                            for qc, (q0, qw) in enumerate(PCH):
                                transpose(psm[:kw, q0:q0 + qw],
                                          Pn[(hh, q0)][:qw, k0:k0 + kw])
                            pts = pst.tile([128, P], BF16, tag="PTs")
                            nc.vector.tensor_copy(pts[:kw, :], psm[:kw, :P])
                            PTs.append(pts)
                        ps_o = pool_sm.tile([128, 200], F32, tag="ss",
                                            bufs=3)
                        for kc, (k0, kw) in enumerate(PCH):
                            nc.tensor.matmul(
                                ps_o[:64, :P],
                                vNc[kc][:kw, hoff:hoff + 64],
                                PTs[kc][:kw, :],
                                start=(kc == 0), stop=(kc == 1))
                        nc.vector.tensor_copy(
                            saIn[mi][hoff:hoff + 64, boff:boff + P],
                            ps_o[:64, :P])
            saT = []
            for m in range(2):
                pmm = pool_mm.tile([128, 512], F32, tag="mm")
                for ki in range(2):
                    nc.tensor.matmul(pmm[:, :2 * P],
                                     opWT[ki][:, m * 128:(m + 1) * 128],
                                     saIn[ki][:],
                                     start=(ki == 0), stop=(ki == 1))
                st = sb1.tile([128, 2 * P], BF16, tag=f"saT{m}", name=f"saT{m}")
                nc.scalar.activation(st[:], pmm[:, :2 * P], AF.Identity,
                                     bias=opb[:, m:m + 1])
                saT.append(st)

            # ---- phases 3+4 per batch item ----
            for b01 in range(2):
                b = pair * 2 + b01
                boff = b01 * P
                # cross-attention: logits transposed L^T[t, p] so the
                # exp eviction directly yields the P'^T matmul operand
                PcT = []
                for tc in range(8):
                    plt = pool_mm.tile([128, 512], F32, tag="mm")
                    for ki in range(2):
                        nc.tensor.matmul(
                            plt[:, :P],
                            tokT[(b01, ki)][:, tc * 128:(tc + 1) * 128],
                            saT[ki][:, boff:boff + P],
                            start=(ki == 0), stop=(ki == 1))
                    pct = sb1.tile([128, P], BF16, tag=f"PcT{tc}",
                                   name=f"PcT{tc}")
                    nc.scalar.activation(pct[:], plt[:, :P], AF.Exp,
                                         bias=0.0, scale=CROSS_SCALE)
                    PcT.append(pct)
                # cross out; tokN's ones column makes psum col 256 = sum(exp)
                poN = []
                for pc, (p0, pw) in enumerate(PCH):
                    pco = pool_mm.tile([128, 512], F32, tag="mm")
                    for tc in range(8):
                        nc.tensor.matmul(pco[:pw, :E + 1],
                                         PcT[tc][:, p0:p0 + pw],
                                         tokN[(b01, tc)][:],
                                         start=(tc == 0), stop=(tc == 7))
                    rr = pst.tile([128, 1], F32, tag="rz")
                    nc.vector.reciprocal(rr[:pw], pco[:pw, E:E + 1])
                    pn = sb1.tile([128, E], BF16, tag=f"poN{pc}",
                                  name=f"poN{pc}")
                    nc.scalar.activation(pn[:pw], pco[:pw, :E], AF.Copy,
                                         bias=0.0, scale=rr[:pw])
                    poN.append(pn)
                # prompt_out^T e-chunks
                poT = []
                for mi in range(2):
                    psm = pool_sm.tile([128, 256], BF16, tag="sm")
                    for pc, (p0, pw) in enumerate(PCH):
                        transpose(psm[:, p0:p0 + pw],
                                  poN[pc][:pw, mi * 128:(mi + 1) * 128])
                    pt = sb1.tile([128, P], BF16, tag=f"poT{mi}",
                                  name=f"poT{mi}")
                    nc.vector.tensor_copy(pt[:], psm[:, :P])
                    poT.append(pt)
                # ---- phase 4: up-proj + gate + store ----
                mtiles = [(poT, 0, 128, 0), (poT, 128, 72, 128)]
                for tt in range(8):
                    mtiles.append((None, tt * 128, 128, P + tt * 128))
                for idx, (src, off, mw, orow) in enumerate(mtiles):
                    outT = pout.tile([128, C], F32, tag="outT")
                    for ncc in range(2):
                        pmm = pool_mm.tile([128, 512], F32, tag="mm")
                        for ki in range(2):
                            lh = (src[ki][:, off:off + mw] if src is not None
                                  else tokT[(b01, ki)][:, off:off + mw])
                            nc.tensor.matmul(
                                pmm[:mw, :],
                                lh, upWT[ki][:, ncc * 512:(ncc + 1) * 512],
                                start=(ki == 0), stop=(ki == 1))
                        if idx % 2 == 0:
                            nc.scalar.activation(
                                outT[:mw, ncc * 512:(ncc + 1) * 512],
                                pmm[:mw, :], AF.Copy, bias=0.0,
                                scale=g128[:mw])
                        else:
                            nc.vector.tensor_scalar_mul(
                                outT[:mw, ncc * 512:(ncc + 1) * 512],
                                pmm[:mw, :], g128[:mw])
                    nc.gpsimd.dma_start(out_d[b, orow:orow + mw, :], outT[:mw, :])

    nc.compile()
    return nc


_NC = None


def _get_nc():
    global _NC
    if _NC is None:
        _NC = build_nc()
    return _NC


def _prep_consts(down_W, down_b, up_W, up_b, in_proj_W, in_proj_b,
                 out_proj_W, out_proj_b, gate):
    f = np.float32
    down_W = np.asarray(down_W, f)
    in_proj_W = np.asarray(in_proj_W, f).copy()
    in_proj_b = np.asarray(in_proj_b, f).copy()
    in_proj_W[:E] *= ATT_SCALE
    in_proj_b[:E] *= ATT_SCALE
    dwT = np.ascontiguousarray(
        down_W.T.reshape(8, 128, E).transpose(1, 0, 2).reshape(128, 2048))
    ipwt = np.ascontiguousarray(in_proj_W.T.reshape(2, 128, 768))
    opwt = np.ascontiguousarray(
        np.asarray(out_proj_W, f).T.reshape(2, 128, 256))
    upwt = np.ascontiguousarray(
        np.asarray(up_W, f).T.reshape(2, 128, 1024))
    return {
        "dwT": dwT, "ipwt": ipwt, "opwt": opwt, "upwt": upwt,
        "ident": np.eye(128, dtype=f),
        "dbias": np.ascontiguousarray(np.asarray(down_b, f).reshape(2, 128).T),
        "qkvb": np.ascontiguousarray(in_proj_b.reshape(6, 128).T),
        "opb": np.ascontiguousarray(np.asarray(out_proj_b, f).reshape(2, 128).T),
        "g128": np.full((128, 1), np.float32(np.asarray(gate))),
    }


def run_kernel(inputs, trace=False):
    """Build in_maps, run on 8 cores, return (full_output, BassKernelResults)."""
    x = np.asarray(inputs["x"], np.float32)
    xb = np.zeros((B, NPAD, C), dtype=ml_dtypes.bfloat16)
    xb[:, :NTOK] = x.astype(ml_dtypes.bfloat16)
    consts = _prep_consts(
        inputs["down_W"], inputs["down_b"], inputs["up_W"], inputs["up_b"],
        inputs["in_proj_W"], inputs["in_proj_b"], inputs["out_proj_W"],
        inputs["out_proj_b"], inputs["gate"])
    in_maps = [dict(xb=xb[c * BL:(c + 1) * BL], **consts)
               for c in range(NCORES)]
    nc = _get_nc()
    res = run_bass_kernel_spmd(nc, in_maps, core_ids=list(range(NCORES)),
                               trace=trace)
    out = np.concatenate([res.results[i]["out"] for i in range(NCORES)], axis=0)
    up_b = np.asarray(inputs["up_b"], np.float32)
    gate = np.float32(np.asarray(inputs["gate"]))
    if np.any(up_b):
        out = out + gate * up_b
    return out, res


def kernel(**inputs):
    out, _ = run_kernel(inputs, trace=False)
    return out


# revision 10
# speedup vs baseline: 1.1587x; 1.1587x over previous
"""Trainium2 Bass kernel for nn_Adapter_SelfParam_CrossNonParam.

Bottleneck adapter: down-proj(1024->256)+exact GELU, self-attention over
the first 200 prompt tokens (4 heads), parameter-free cross-attention
prompt->tokens, concat, up-proj(256->1024), gate.

Sharding: data-parallel over batch B=64 across 8 NeuronCores (8 items
each); all weights replicated. No collectives.

V2: full bf16 datapath (operand rounding ~2e-3, PSUM accumulation fp32).
x arrives as bf16 (padded to 1280 tokens) and is transposed by the DMA
xbar engines straight from DRAM into feature-major SBUF tiles - no PE
transposes for x. All intermediates are feature-major so matmul
contractions land on partitions; the few small transposes (tok-natural,
P^T, v, P'^T, prompt_out^T) run on the PE at 1 cyc/row in bf16.
"""
import sys

sys.path.insert(0, "/opt/trn_rl_repo")

import numpy as np
import ml_dtypes
from contextlib import ExitStack

import concourse.bass as bass
import concourse.tile as tile
from concourse import bacc, mybir
from concourse.bass_utils import run_bass_kernel_spmd

F32 = mybir.dt.float32
BF16 = mybir.dt.bfloat16
AF = mybir.ActivationFunctionType

B, NTOK, C = 64, 1224, 1024
NPAD = 1280                 # token dim padded for DMA-transpose alignment
E, P, T = 256, 200, 1024
NH, HD = 4, 64
NCORES, BL = 8, 8           # cores, batch per core
ATT_SCALE = 1.0 / np.sqrt(HD)   # folded into q weights host-side
CROSS_SCALE = float(E) ** -0.5  # folded into cross-softmax exp scale

# prompt chunks (rows of the 200-token prompt)
PCH = [(0, 128), (128, 72)]
# down-proj N chunks
DNCH = [(0, 512), (512, 456), (968, 256)]


def build_nc():
    nc = bacc.Bacc("TRN2", target_bir_lowering=False, debug=False,
                   num_devices=NCORES)

    x_d = nc.dram_tensor("xb", [BL, NPAD, C], BF16, kind="ExternalInput").ap()
    dwT_d = nc.dram_tensor("dwT", [128, 2048], F32, kind="ExternalInput").ap()
    ipwt_d = nc.dram_tensor("ipwt", [2, 128, 768], F32, kind="ExternalInput").ap()
    opwt_d = nc.dram_tensor("opwt", [2, 128, 256], F32, kind="ExternalInput").ap()
    upwt_d = nc.dram_tensor("upwt", [2, 128, 1024], F32, kind="ExternalInput").ap()
    ident_d = nc.dram_tensor("ident", [128, 128], F32, kind="ExternalInput").ap()
    dbias_d = nc.dram_tensor("dbias", [128, 2], F32, kind="ExternalInput").ap()
    qkvb_d = nc.dram_tensor("qkvb", [128, 6], F32, kind="ExternalInput").ap()
    opb_d = nc.dram_tensor("opb", [128, 2], F32, kind="ExternalInput").ap()
    g_d = nc.dram_tensor("g128", [128, 1], F32, kind="ExternalInput").ap()
    out_d = nc.dram_tensor("out", [BL, NTOK, C], F32, kind="ExternalOutput").ap()

    with tile.TileContext(nc) as tc, ExitStack() as ctx:
        wp = ctx.enter_context(tc.tile_pool(name="wts", bufs=1))
        sb1 = ctx.enter_context(tc.tile_pool(name="sb1", bufs=1))
        sbx = ctx.enter_context(tc.tile_pool(name="sbx", bufs=2))
        pst = ctx.enter_context(tc.tile_pool(name="pst", bufs=8))
        pout = ctx.enter_context(tc.tile_pool(name="pout", bufs=3))
        pool_mm = ctx.enter_context(tc.tile_pool(name="pmm", bufs=3, space="PSUM"))
        pool_sm = ctx.enter_context(tc.tile_pool(name="psm", bufs=2, space="PSUM"))

        # ---- resident weights (bf16 via gpsimd cast-DMA) ----
        dwT = wp.tile([128, 2048], BF16, tag="dwT")
        nc.gpsimd.dma_start(dwT[:], dwT_d[:])
        ipWT = []
        for ki in range(2):
            t = wp.tile([128, 768], BF16, tag=f"ipWT{ki}", name=f"ipWT{ki}")
            nc.gpsimd.dma_start(t[:], ipwt_d[ki])
            ipWT.append(t)
        opWT = []
        for ki in range(2):
            t = wp.tile([128, 256], BF16, tag=f"opWT{ki}", name=f"opWT{ki}")
            nc.gpsimd.dma_start(t[:], opwt_d[ki])
            opWT.append(t)
        upWT = []
        for ki in range(2):
            t = wp.tile([128, 1024], BF16, tag=f"upWT{ki}", name=f"upWT{ki}")
            nc.gpsimd.dma_start(t[:], upwt_d[ki])
            upWT.append(t)
        idB = wp.tile([128, 128], BF16, tag="idB")
        nc.gpsimd.dma_start(idB[:], ident_d[:])
        dbias = wp.tile([128, 2], F32, tag="dbias")
        nc.sync.dma_start(dbias[:], dbias_d[:])
        qkvb = wp.tile([128, 6], F32, tag="qkvb")
        nc.sync.dma_start(qkvb[:], qkvb_d[:])
        opb = wp.tile([128, 2], F32, tag="opb")
        nc.sync.dma_start(opb[:], opb_d[:])
        g128 = wp.tile([128, 1], F32, tag="g128")
        nc.sync.dma_start(g128[:], g_d[:])

        # prompt^T for all 8 batch items: [256 feat, 8*200]
        promT = [wp.tile([128, BL * P], BF16, tag=f"promT{m}", name=f"promT{m}")
                 for m in range(2)]

        def transpose(out_ap, in_ap):
            pw = in_ap.partition_size()
            bp = in_ap.base_partition()
            nc.tensor.transpose(out_ap, in_ap, idB[bp:bp + pw, bp:bp + pw])

        tokT = {}   # (b, m)  -> [128, 1024] tok^T e-chunk
        tokN = {}   # (b, tt) -> [128, 257] tok natural + ones col

        # ================= PHASE A: down-proj + gelu (all b) ============
        for b in range(BL):
            xT = [sbx.tile([128, NPAD], BF16, tag=f"xT{ci}",
                           name=f"xT{ci}") for ci in range(8)]
            for ci in range(8):
                nc.sync.dma_start(out=xT[ci][:],
                                  in_=x_d[b, :, ci * 128:(ci + 1) * 128],
                                  transpose=True)
            for m in range(2):
                tokTm = sb1.tile([128, T], BF16, tag=f"tokT{b}_{m}",
                                 name=f"tokT{b}_{m}")
                tokT[(b, m)] = tokTm
                for (n0, nw) in DNCH:
                    pmm = pool_mm.tile([128, 512], F32, tag="mm")
                    for ci in range(8):
                        nc.tensor.matmul(
                            pmm[:, :nw],
                            dwT[:, ci * 256 + m * 128:ci * 256 + (m + 1) * 128],
                            xT[ci][:, n0:n0 + nw],
                            start=(ci == 0), stop=(ci == 7))
                    if n0 == 0:
                        nc.scalar.activation(
                            promT[m][:, b * P:(b + 1) * P], pmm[:, :P],
                            AF.Gelu, bias=dbias[:, m:m + 1])
                        nc.scalar.activation(
                            tokTm[:, 0:nw - P], pmm[:, P:nw],
                            AF.Gelu, bias=dbias[:, m:m + 1])
                    else:
                        nc.scalar.activation(
                            tokTm[:, n0 - P:n0 - P + nw], pmm[:, :nw],
                            AF.Gelu, bias=dbias[:, m:m + 1])
            # tok natural layout (+ ones col for the cross softmax denom)
            for tt in range(8):
                psm = pool_sm.tile([128, 256], BF16, tag="sm")
                for m in range(2):
                    transpose(psm[:, m * 128:(m + 1) * 128],
                              tokT[(b, m)][:, tt * 128:(tt + 1) * 128])
                tokNt = sb1.tile([128, 257], BF16, tag=f"tokN{b}_{tt}",
                                 name=f"tokN{b}_{tt}")
                nc.vector.tensor_copy(tokNt[:, :256], psm[:])
                nc.vector.memset(tokNt[:, 256:257], 1.0)
                tokN[(b, tt)] = tokNt

        # ================= PHASE B: attention + cross + up ==============
        # qkv for all 8 prompts at once: [768 feat, 1600]
        qkvT = []
        for mi in range(6):
            qt = sb1.tile([128, BL * P], BF16, tag=f"qkvT{mi}",
                          name=f"qkvT{mi}")
            for nq in range(4):
                pmm = pool_mm.tile([128, 512], F32, tag="mm")
                for ki in range(2):
                    nc.tensor.matmul(
                        pmm[:, :400],
                        ipWT[ki][:, mi * 128:(mi + 1) * 128],
                        promT[ki][:, nq * 400:(nq + 1) * 400],
                        start=(ki == 0), stop=(ki == 1))
                nc.scalar.activation(qt[:, nq * 400:(nq + 1) * 400],
                                     pmm[:, :400], AF.Identity,
                                     bias=qkvb[:, mi:mi + 1])
            qkvT.append(qt)

        for b in range(BL):
            boff = b * P
            # ---- self-attention ----
            saIn = [sb1.tile([128, P], BF16, tag=f"saIn{ki}",
                             name=f"saIn{ki}", bufs=2) for ki in range(2)]
            for mi in range(2):
                qv, kv, vv = qkvT[mi], qkvT[2 + mi], qkvT[4 + mi]
                vNc = []
                for (k0, kw) in PCH:
                    psv = pool_sm.tile([128, 256], BF16, tag="sm")
                    transpose(psv[:kw, :128],
                              vv[:, boff + k0:boff + k0 + kw])
                    vn = pst.tile([128, 128], BF16, tag="vNc")
                    nc.vector.tensor_copy(vn[:kw, :], psv[:kw, :128])
                    vNc.append(vn)
                Pn = {}
                for (q0, qw) in PCH:
                    pss = []
                    for hh in range(2):
                        hoff = hh * 64
                        ps = pool_sm.tile([128, 200], F32, tag="ss", bufs=3)
                        nc.tensor.matmul(
                            ps[:qw, :P],
                            qv[hoff:hoff + 64, boff + q0:boff + q0 + qw],
                            kv[hoff:hoff + 64, boff:boff + P],
                            start=True, stop=True)
                        pss.append(ps)
                    for hh in range(2):
                        pe = pst.tile([128, P], BF16, tag="Pexp")
                        se = pst.tile([128, 1], F32, tag="sexp")
                        nc.scalar.activation(pe[:qw], pss[hh][:qw, :P],
                                             AF.Exp, bias=0.0,
                                             accum_out=se[:qw])
                        rc = pst.tile([128, 1], F32, tag="recip")
                        nc.vector.reciprocal(rc[:qw], se[:qw])
                        pn = pst.tile([128, P], BF16, tag="Pn")
                        nc.vector.tensor_scalar_mul(pn[:qw], pe[:qw], rc[:qw])
                        Pn[(hh, q0)] = pn
                for hh in range(2):
                    hoff = hh * 64
                    PTs = []
                    for (k0, kw) in PCH:
                        psm = pool_sm.tile([128, 256], BF16, tag="sm")
                        for qc, (q0, qw) in enumerate(PCH):
                            transpose(psm[:kw, q0:q0 + qw],
                                      Pn[(hh, q0)][:qw, k0:k0 + kw])
                        pts = pst.tile([128, P], BF16, tag="PTs")
                        nc.vector.tensor_copy(pts[:kw, :], psm[:kw, :P])
                        PTs.append(pts)
                    ps_o = pool_sm.tile([128, 200], F32, tag="ss", bufs=3)
                    for kc, (k0, kw) in enumerate(PCH):
                        nc.tensor.matmul(
                            ps_o[:64, :P],
                            vNc[kc][:kw, hoff:hoff + 64],
                            PTs[kc][:kw, :],
                            start=(kc == 0), stop=(kc == 1))
                    nc.vector.tensor_copy(saIn[mi][hoff:hoff + 64, :],
                                          ps_o[:64, :P])
            # ---- out_proj ----
            saT = []
            for m in range(2):
                pmm = pool_mm.tile([128, 512], F32, tag="mm")
                for ki in range(2):
                    nc.tensor.matmul(pmm[:, :P],
                                     opWT[ki][:, m * 128:(m + 1) * 128],
                                     saIn[ki][:],
                                     start=(ki == 0), stop=(ki == 1))
                st = sb1.tile([128, P], BF16, tag=f"saT{m}", name=f"saT{m}",
                              bufs=2)
                nc.scalar.activation(st[:], pmm[:, :P], AF.Identity,
                                     bias=opb[:, m:m + 1])
                saT.append(st)
            # ---- cross-attention: logits transposed L^T[t, p] ----
            PcT = []
            for tc in range(8):
                plt = pool_mm.tile([128, 512], F32, tag="mm")
                for ki in range(2):
                    nc.tensor.matmul(
                        plt[:, :P],
                        tokT[(b, ki)][:, tc * 128:(tc + 1) * 128],
                        saT[ki][:],
                        start=(ki == 0), stop=(ki == 1))
                pct = sb1.tile([128, P], BF16, tag=f"PcT{tc}",
                               name=f"PcT{tc}", bufs=2)
                nc.scalar.activation(pct[:], plt[:, :P], AF.Exp,
                                     bias=0.0, scale=CROSS_SCALE)
                PcT.append(pct)
            # cross out; tokN's ones column makes psum col 256 = sum(exp)
            poN = []
            for pc, (p0, pw) in enumerate(PCH):
                pco = pool_mm.tile([128, 512], F32, tag="mm")
                for tc in range(8):
                    nc.tensor.matmul(pco[:pw, :E + 1],
                                     PcT[tc][:, p0:p0 + pw],
                                     tokN[(b, tc)][:],
                                     start=(tc == 0), stop=(tc == 7))
                rr = pst.tile([128, 1], F32, tag="rz")
                nc.vector.reciprocal(rr[:pw], pco[:pw, E:E + 1])
                pn = sb1.tile([128, E], BF16, tag=f"poN{pc}",
                              name=f"poN{pc}", bufs=2)
                nc.scalar.activation(pn[:pw], pco[:pw, :E], AF.Copy,
                                     bias=0.0, scale=rr[:pw])
                poN.append(pn)
            # prompt_out^T e-chunks
            poT = []
            for mi in range(2):
                psm = pool_sm.tile([128, 256], BF16, tag="sm")
                for pc, (p0, pw) in enumerate(PCH):
                    transpose(psm[:, p0:p0 + pw],
                              poN[pc][:pw, mi * 128:(mi + 1) * 128])
                pt = sb1.tile([128, P], BF16, tag=f"poT{mi}",
                              name=f"poT{mi}", bufs=2)
                nc.vector.tensor_copy(pt[:], psm[:, :P])
                poT.append(pt)
            # ---- up-proj + gate + store ----
            mtiles = [(poT, 0, 128, 0), (poT, 128, 72, 128)]
            for tt in range(8):
                mtiles.append((None, tt * 128, 128, P + tt * 128))
            for idx, (src, off, mw, orow) in enumerate(mtiles):
                outT = pout.tile([128, C], F32, tag="outT")
                for ncc in range(2):
                    pmm = pool_mm.tile([128, 512], F32, tag="mm")
                    for ki in range(2):
                        lh = (src[ki][:, off:off + mw] if src is not None
                              else tokT[(b, ki)][:, off:off + mw])
                        nc.tensor.matmul(
                            pmm[:mw, :],
                            lh, upWT[ki][:, ncc * 512:(ncc + 1) * 512],
                            start=(ki == 0), stop=(ki == 1))
                    if idx % 2 == 0:
                        nc.scalar.activation(
                            outT[:mw, ncc * 512:(ncc + 1) * 512],
                            pmm[:mw, :], AF.Copy, bias=0.0,
                            scale=g128[:mw])
                    else:
                        nc.vector.tensor_scalar_mul(
                            outT[:mw, ncc * 512:(ncc + 1) * 512],
                            pmm[:mw, :], g128[:mw])
                nc.gpsimd.dma_start(out_d[b, orow:orow + mw, :], outT[:mw, :])

    nc.compile()
    return nc


_NC = None


def _get_nc():
    global _NC
    if _NC is None:
        _NC = build_nc()
    return _NC


def _prep_consts(down_W, down_b, up_W, up_b, in_proj_W, in_proj_b,
                 out_proj_W, out_proj_b, gate):
    f = np.float32
    down_W = np.asarray(down_W, f)
    in_proj_W = np.asarray(in_proj_W, f).copy()
    in_proj_b = np.asarray(in_proj_b, f).copy()
    in_proj_W[:E] *= ATT_SCALE
    in_proj_b[:E] *= ATT_SCALE
    dwT = np.ascontiguousarray(
        down_W.T.reshape(8, 128, E).transpose(1, 0, 2).reshape(128, 2048))
    ipwt = np.ascontiguousarray(in_proj_W.T.reshape(2, 128, 768))
    opwt = np.ascontiguousarray(
        np.asarray(out_proj_W, f).T.reshape(2, 128, 256))
    upwt = np.ascontiguousarray(
        np.asarray(up_W, f).T.reshape(2, 128, 1024))
    return {
        "dwT": dwT, "ipwt": ipwt, "opwt": opwt, "upwt": upwt,
        "ident": np.eye(128, dtype=f),
        "dbias": np.ascontiguousarray(np.asarray(down_b, f).reshape(2, 128).T),
        "qkvb": np.ascontiguousarray(in_proj_b.reshape(6, 128).T),
        "opb": np.ascontiguousarray(np.asarray(out_proj_b, f).reshape(2, 128).T),
        "g128": np.full((128, 1), np.float32(np.asarray(gate))),
    }


def run_kernel(inputs, trace=False):
    """Build in_maps, run on 8 cores, return (full_output, BassKernelResults)."""
    x = np.asarray(inputs["x"], np.float32)
    xb = np.zeros((B, NPAD, C), dtype=ml_dtypes.bfloat16)
    xb[:, :NTOK] = x.astype(ml_dtypes.bfloat16)
    consts = _prep_consts(
        inputs["down_W"], inputs["down_b"], inputs["up_W"], inputs["up_b"],
        inputs["in_proj_W"], inputs["in_proj_b"], inputs["out_proj_W"],
        inputs["out_proj_b"], inputs["gate"])
    in_maps = [dict(xb=xb[c * BL:(c + 1) * BL], **consts)
               for c in range(NCORES)]
    nc = _get_nc()
    res = run_bass_kernel_spmd(nc, in_maps, core_ids=list(range(NCORES)),
                               trace=trace)
    out = np.concatenate([res.results[i]["out"] for i in range(NCORES)], axis=0)
    up_b = np.asarray(inputs["up_b"], np.float32)
    gate = np.float32(np.asarray(inputs["gate"]))
    if np.any(up_b):
        out = out + gate * up_b
    return out, res


def kernel(**inputs):
    out, _ = run_kernel(inputs, trace=False)
    return out


# revision 11
# speedup vs baseline: 1.1779x; 1.0165x over previous
"""Trainium2 Bass kernel for nn_Adapter_SelfParam_CrossNonParam.

Bottleneck adapter: down-proj(1024->256)+exact GELU, self-attention over
the first 200 prompt tokens (4 heads), parameter-free cross-attention
prompt->tokens, concat, up-proj(256->1024), gate.

Sharding: data-parallel over batch B=64 across 8 NeuronCores (8 items
each); all weights replicated. No collectives.

V2: full bf16 datapath (operand rounding ~2e-3, PSUM accumulation fp32).
x arrives as bf16 (padded to 1280 tokens) and is transposed by the DMA
xbar engines straight from DRAM into feature-major SBUF tiles - no PE
transposes for x. All intermediates are feature-major so matmul
contractions land on partitions; the few small transposes (tok-natural,
P^T, v, P'^T, prompt_out^T) run on the PE at 1 cyc/row in bf16.
"""
import sys

sys.path.insert(0, "/opt/trn_rl_repo")

import numpy as np
import ml_dtypes
from contextlib import ExitStack

import concourse.bass as bass
import concourse.tile as tile
from concourse import bacc, mybir
from concourse.bass_utils import run_bass_kernel_spmd

F32 = mybir.dt.float32
BF16 = mybir.dt.bfloat16
AF = mybir.ActivationFunctionType

B, NTOK, C = 64, 1224, 1024
NPAD = 1280                 # token dim padded for DMA-transpose alignment
E, P, T = 256, 200, 1024
NH, HD = 4, 64
NCORES, BL = 8, 8           # cores, batch per core
ATT_SCALE = 1.0 / np.sqrt(HD)   # folded into q weights host-side
CROSS_SCALE = float(E) ** -0.5  # folded into cross-softmax exp scale

# prompt chunks (rows of the 200-token prompt)
PCH = [(0, 128), (128, 72)]
# down-proj N chunks
DNCH = [(0, 512), (512, 456), (968, 256)]


def build_nc():
    nc = bacc.Bacc("TRN2", target_bir_lowering=False, debug=False,
                   num_devices=NCORES)

    x_d = nc.dram_tensor("xb", [BL, 8, NPAD, 128], BF16, kind="ExternalInput").ap()
    dwT_d = nc.dram_tensor("dwT", [128, 2048], F32, kind="ExternalInput").ap()
    ipwt_d = nc.dram_tensor("ipwt", [2, 128, 768], F32, kind="ExternalInput").ap()
    opwt_d = nc.dram_tensor("opwt", [2, 128, 256], F32, kind="ExternalInput").ap()
    upwt_d = nc.dram_tensor("upwt", [2, 128, 1024], F32, kind="ExternalInput").ap()
    ident_d = nc.dram_tensor("ident", [128, 128], F32, kind="ExternalInput").ap()
    dbias_d = nc.dram_tensor("dbias", [128, 2], F32, kind="ExternalInput").ap()
    qkvb_d = nc.dram_tensor("qkvb", [128, 6], F32, kind="ExternalInput").ap()
    opb_d = nc.dram_tensor("opb", [128, 2], F32, kind="ExternalInput").ap()
    g_d = nc.dram_tensor("g128", [128, 1], F32, kind="ExternalInput").ap()
    out_d = nc.dram_tensor("out", [BL, NTOK, C], F32, kind="ExternalOutput").ap()

    with tile.TileContext(nc) as tc, ExitStack() as ctx:
        wp = ctx.enter_context(tc.tile_pool(name="wts", bufs=1))
        sb1 = ctx.enter_context(tc.tile_pool(name="sb1", bufs=1))
        sbx = ctx.enter_context(tc.tile_pool(name="sbx", bufs=2))
        pst = ctx.enter_context(tc.tile_pool(name="pst", bufs=8))
        pout = ctx.enter_context(tc.tile_pool(name="pout", bufs=3))
        pool_mm = ctx.enter_context(tc.tile_pool(name="pmm", bufs=3, space="PSUM"))
        pool_sm = ctx.enter_context(tc.tile_pool(name="psm", bufs=3, space="PSUM"))

        # ---- resident weights (bf16 via gpsimd cast-DMA) ----
        dwT = wp.tile([128, 2048], BF16, tag="dwT")
        nc.gpsimd.dma_start(dwT[:], dwT_d[:])
        ipWT = []
        for ki in range(2):
            t = wp.tile([128, 768], BF16, tag=f"ipWT{ki}", name=f"ipWT{ki}")
            nc.gpsimd.dma_start(t[:], ipwt_d[ki])
            ipWT.append(t)
        opWT = []
        for ki in range(2):
            t = wp.tile([128, 256], BF16, tag=f"opWT{ki}", name=f"opWT{ki}")
            nc.gpsimd.dma_start(t[:], opwt_d[ki])
            opWT.append(t)
        upWT = []
        for ki in range(2):
            t = wp.tile([128, 1024], BF16, tag=f"upWT{ki}", name=f"upWT{ki}")
            nc.gpsimd.dma_start(t[:], upwt_d[ki])
            upWT.append(t)
        idB = wp.tile([128, 128], BF16, tag="idB")
        nc.gpsimd.dma_start(idB[:], ident_d[:])
        dbias = wp.tile([128, 2], F32, tag="dbias")
        nc.sync.dma_start(dbias[:], dbias_d[:])
        qkvb = wp.tile([128, 6], F32, tag="qkvb")
        nc.sync.dma_start(qkvb[:], qkvb_d[:])
        opb = wp.tile([128, 2], F32, tag="opb")
        nc.sync.dma_start(opb[:], opb_d[:])
        g128 = wp.tile([128, 1], F32, tag="g128")
        nc.sync.dma_start(g128[:], g_d[:])

        # prompt^T for all 8 batch items: [256 feat, 8*200]
        promT = [wp.tile([128, BL * P], BF16, tag=f"promT{m}", name=f"promT{m}")
                 for m in range(2)]

        def transpose(out_ap, in_ap):
            pw = in_ap.partition_size()
            bp = in_ap.base_partition()
            nc.tensor.transpose(out_ap, in_ap, idB[bp:bp + pw, bp:bp + pw])

        tokT = {}   # (b, m)  -> [128, 1024] tok^T e-chunk
        tokN = {}   # (b, tt) -> [128, 257] tok natural + ones col

        # ================= PHASE A: down-proj + gelu (all b) ============
        for b in range(BL):
            xT = [sbx.tile([128, NPAD], BF16, tag=f"xT{ci}",
                           name=f"xT{ci}") for ci in range(8)]
            for ci in range(8):
                nc.sync.dma_start(out=xT[ci][:], in_=x_d[b, ci],
                                  transpose=True)
            for m in range(2):
                tokTm = sb1.tile([128, T], BF16, tag=f"tokT{b}_{m}",
                                 name=f"tokT{b}_{m}")
                tokT[(b, m)] = tokTm
                for (n0, nw) in DNCH:
                    pmm = pool_mm.tile([128, 512], F32, tag="mm")
                    for ci in range(8):
                        nc.tensor.matmul(
                            pmm[:, :nw],
                            dwT[:, ci * 256 + m * 128:ci * 256 + (m + 1) * 128],
                            xT[ci][:, n0:n0 + nw],
                            start=(ci == 0), stop=(ci == 7))
                    if n0 == 0:
                        nc.scalar.activation(
                            promT[m][:, b * P:(b + 1) * P], pmm[:, :P],
                            AF.Gelu, bias=dbias[:, m:m + 1])
                        nc.scalar.activation(
                            tokTm[:, 0:nw - P], pmm[:, P:nw],
                            AF.Gelu, bias=dbias[:, m:m + 1])
                    else:
                        nc.scalar.activation(
                            tokTm[:, n0 - P:n0 - P + nw], pmm[:, :nw],
                            AF.Gelu, bias=dbias[:, m:m + 1])
            # tok natural layout (+ ones col for the cross softmax denom)
            for tt in range(8):
                psm = pool_sm.tile([128, 256], BF16, tag="sm")
                for m in range(2):
                    transpose(psm[:, m * 128:(m + 1) * 128],
                              tokT[(b, m)][:, tt * 128:(tt + 1) * 128])
                tokNt = sb1.tile([128, 257], BF16, tag=f"tokN{b}_{tt}",
                                 name=f"tokN{b}_{tt}")
                nc.vector.tensor_copy(tokNt[:, :256], psm[:])
                nc.vector.memset(tokNt[:, 256:257], 1.0)
                tokN[(b, tt)] = tokNt

        # ================= PHASE B: attention + cross + up ==============
        # qkv for all 8 prompts at once: [768 feat, 1600]
        qkvT = []
        for mi in range(6):
            qt = sb1.tile([128, BL * P], BF16, tag=f"qkvT{mi}",
                          name=f"qkvT{mi}")
            for nq in range(4):
                pmm = pool_mm.tile([128, 512], F32, tag="mm")
                for ki in range(2):
                    nc.tensor.matmul(
                        pmm[:, :400],
                        ipWT[ki][:, mi * 128:(mi + 1) * 128],
                        promT[ki][:, nq * 400:(nq + 1) * 400],
                        start=(ki == 0), stop=(ki == 1))
                nc.vector.tensor_scalar_add(qt[:, nq * 400:(nq + 1) * 400],
                                            pmm[:, :400], qkvb[:, mi:mi + 1])
            qkvT.append(qt)

        for b in range(BL):
            boff = b * P
            # ---- self-attention ----
            saIn = [sb1.tile([128, P], BF16, tag=f"saIn{ki}",
                             name=f"saIn{ki}", bufs=2) for ki in range(2)]
            for mi in range(2):
                qv, kv, vv = qkvT[mi], qkvT[2 + mi], qkvT[4 + mi]
                vNc = []
                for (k0, kw) in PCH:
                    psv = pool_sm.tile([128, 256], BF16, tag="sm")
                    transpose(psv[:kw, :128],
                              vv[:, boff + k0:boff + k0 + kw])
                    vn = pst.tile([128, 128], BF16, tag="vNc")
                    nc.vector.tensor_copy(vn[:kw, :], psv[:kw, :128])
                    vNc.append(vn)
                Pn = {}
                for (q0, qw) in PCH:
                    pss = []
                    for hh in range(2):
                        hoff = hh * 64
                        ps = pool_sm.tile([128, 200], F32, tag="ss", bufs=2)
                        nc.tensor.matmul(
                            ps[:qw, :P],
                            qv[hoff:hoff + 64, boff + q0:boff + q0 + qw],
                            kv[hoff:hoff + 64, boff:boff + P],
                            start=True, stop=True)
                        pss.append(ps)
                    for hh in range(2):
                        pe = pst.tile([128, P], BF16, tag="Pexp")
                        se = pst.tile([128, 1], F32, tag="sexp")
                        nc.scalar.activation(pe[:qw], pss[hh][:qw, :P],
                                             AF.Exp, bias=0.0,
                                             accum_out=se[:qw])
                        rc = pst.tile([128, 1], F32, tag="recip")
                        nc.vector.reciprocal(rc[:qw], se[:qw])
                        pn = pst.tile([128, P], BF16, tag="Pn")
                        nc.vector.tensor_scalar_mul(pn[:qw], pe[:qw], rc[:qw])
                        Pn[(hh, q0)] = pn
                for hh in range(2):
                    hoff = hh * 64
                    PTs = []
                    for (k0, kw) in PCH:
                        psm = pool_sm.tile([128, 256], BF16, tag="sm")
                        for qc, (q0, qw) in enumerate(PCH):
                            transpose(psm[:kw, q0:q0 + qw],
                                      Pn[(hh, q0)][:qw, k0:k0 + kw])
                        pts = pst.tile([128, P], BF16, tag="PTs")
                        nc.vector.tensor_copy(pts[:kw, :], psm[:kw, :P])
                        PTs.append(pts)
                    ps_o = pool_sm.tile([128, 200], F32, tag="ss", bufs=2)
                    for kc, (k0, kw) in enumerate(PCH):
                        nc.tensor.matmul(
                            ps_o[:64, :P],
                            vNc[kc][:kw, hoff:hoff + 64],
                            PTs[kc][:kw, :],
                            start=(kc == 0), stop=(kc == 1))
                    nc.vector.tensor_copy(saIn[mi][hoff:hoff + 64, :],
                                          ps_o[:64, :P])
            # ---- out_proj ----
            saT = []
            for m in range(2):
                pmm = pool_mm.tile([128, 512], F32, tag="mm")
                for ki in range(2):
                    nc.tensor.matmul(pmm[:, :P],
                                     opWT[ki][:, m * 128:(m + 1) * 128],
                                     saIn[ki][:],
                                     start=(ki == 0), stop=(ki == 1))
                st = sb1.tile([128, P], BF16, tag=f"saT{m}", name=f"saT{m}",
                              bufs=2)
                nc.vector.tensor_scalar_add(st[:], pmm[:, :P],
                                            opb[:, m:m + 1])
                saT.append(st)
            # ---- cross-attention: logits transposed L^T[t, p] ----
            PcT = []
            for tc in range(8):
                plt = pool_mm.tile([128, 512], F32, tag="mm")
                for ki in range(2):
                    nc.tensor.matmul(
                        plt[:, :P],
                        tokT[(b, ki)][:, tc * 128:(tc + 1) * 128],
                        saT[ki][:],
                        start=(ki == 0), stop=(ki == 1))
                pct = sb1.tile([128, P], BF16, tag=f"PcT{tc}",
                               name=f"PcT{tc}", bufs=2)
                nc.scalar.activation(pct[:], plt[:, :P], AF.Exp,
                                     bias=0.0, scale=CROSS_SCALE)
                PcT.append(pct)
            # cross out; tokN's ones column makes psum col 256 = sum(exp)
            poN = []
            for pc, (p0, pw) in enumerate(PCH):
                pco = pool_mm.tile([128, 512], F32, tag="mm")
                for tc in range(8):
                    nc.tensor.matmul(pco[:pw, :E + 1],
                                     PcT[tc][:, p0:p0 + pw],
                                     tokN[(b, tc)][:],
                                     start=(tc == 0), stop=(tc == 7))
                rr = pst.tile([128, 1], F32, tag="rz")
                nc.vector.reciprocal(rr[:pw], pco[:pw, E:E + 1])
                pn = sb1.tile([128, E], BF16, tag=f"poN{pc}",
                              name=f"poN{pc}", bufs=2)
                nc.scalar.activation(pn[:pw], pco[:pw, :E], AF.Copy,
                                     bias=0.0, scale=rr[:pw])
                poN.append(pn)
            # prompt_out^T e-chunks
            poT = []
            for mi in range(2):
                psm = pool_sm.tile([128, 256], BF16, tag="sm")
                for pc, (p0, pw) in enumerate(PCH):
                    transpose(psm[:, p0:p0 + pw],
                              poN[pc][:pw, mi * 128:(mi + 1) * 128])
                pt = sb1.tile([128, P], BF16, tag=f"poT{mi}",
                              name=f"poT{mi}", bufs=2)
                nc.vector.tensor_copy(pt[:], psm[:, :P])
                poT.append(pt)
            # ---- up-proj + gate + store ----
            mtiles = [(poT, 0, 128, 0), (poT, 128, 72, 128)]
            for tt in range(8):
                mtiles.append((None, tt * 128, 128, P + tt * 128))
            for idx, (src, off, mw, orow) in enumerate(mtiles):
                outT = pout.tile([128, C], F32, tag="outT")
                for ncc in range(2):
                    pmm = pool_mm.tile([128, 512], F32, tag="mm")
                    for ki in range(2):
                        lh = (src[ki][:, off:off + mw] if src is not None
                              else tokT[(b, ki)][:, off:off + mw])
                        nc.tensor.matmul(
                            pmm[:mw, :],
                            lh, upWT[ki][:, ncc * 512:(ncc + 1) * 512],
                            start=(ki == 0), stop=(ki == 1))
                    if idx % 2 == 0:
                        nc.scalar.activation(
                            outT[:mw, ncc * 512:(ncc + 1) * 512],
                            pmm[:mw, :], AF.Copy, bias=0.0,
                            scale=g128[:mw])
                    else:
                        nc.vector.tensor_scalar_mul(
                            outT[:mw, ncc * 512:(ncc + 1) * 512],
                            pmm[:mw, :], g128[:mw])
                nc.gpsimd.dma_start(out_d[b, orow:orow + mw, :], outT[:mw, :])

    nc.compile()
    return nc


_NC = None


def _get_nc():
    global _NC
    if _NC is None:
        _NC = build_nc()
    return _NC


def _prep_consts(down_W, down_b, up_W, up_b, in_proj_W, in_proj_b,
                 out_proj_W, out_proj_b, gate):
    f = np.float32
    down_W = np.asarray(down_W, f)
    in_proj_W = np.asarray(in_proj_W, f).copy()
    in_proj_b = np.asarray(in_proj_b, f).copy()
    in_proj_W[:E] *= ATT_SCALE
    in_proj_b[:E] *= ATT_SCALE
    dwT = np.ascontiguousarray(
        down_W.T.reshape(8, 128, E).transpose(1, 0, 2).reshape(128, 2048))
    ipwt = np.ascontiguousarray(in_proj_W.T.reshape(2, 128, 768))
    opwt = np.ascontiguousarray(
        np.asarray(out_proj_W, f).T.reshape(2, 128, 256))
    upwt = np.ascontiguousarray(
        np.asarray(up_W, f).T.reshape(2, 128, 1024))
    return {
        "dwT": dwT, "ipwt": ipwt, "opwt": opwt, "upwt": upwt,
        "ident": np.eye(128, dtype=f),
        "dbias": np.ascontiguousarray(np.asarray(down_b, f).reshape(2, 128).T),
        "qkvb": np.ascontiguousarray(in_proj_b.reshape(6, 128).T),
        "opb": np.ascontiguousarray(np.asarray(out_proj_b, f).reshape(2, 128).T),
        "g128": np.full((128, 1), np.float32(np.asarray(gate))),
    }


def run_kernel(inputs, trace=False):
    """Build in_maps, run on 8 cores, return (full_output, BassKernelResults)."""
    x = np.asarray(inputs["x"], np.float32)
    xb = np.zeros((B, NPAD, C), dtype=ml_dtypes.bfloat16)
    xb[:, :NTOK] = x.astype(ml_dtypes.bfloat16)
    # chunk feature dim so each (b, ci) transpose-DMA reads contiguous DRAM
    xb = np.ascontiguousarray(
        xb.reshape(B, NPAD, 8, 128).transpose(0, 2, 1, 3))
    consts = _prep_consts(
        inputs["down_W"], inputs["down_b"], inputs["up_W"], inputs["up_b"],
        inputs["in_proj_W"], inputs["in_proj_b"], inputs["out_proj_W"],
        inputs["out_proj_b"], inputs["gate"])
    in_maps = [dict(xb=xb[c * BL:(c + 1) * BL], **consts)
               for c in range(NCORES)]
    nc = _get_nc()
    res = run_bass_kernel_spmd(nc, in_maps, core_ids=list(range(NCORES)),
                               trace=trace)
    out = np.concatenate([res.results[i]["out"] for i in range(NCORES)], axis=0)
    up_b = np.asarray(inputs["up_b"], np.float32)
    gate = np.float32(np.asarray(inputs["gate"]))
    if np.any(up_b):
        out = out + gate * up_b
    return out, res


def kernel(**inputs):
    out, _ = run_kernel(inputs, trace=False)
    return out
